# revision 7
# baseline (speedup 1.0000x reference)
"""Multi-head self-attention with linear relative-position bias on 8 trn2 cores.

Problem: B=4, T=2048, D=1024, H=16 heads (hd=64), fp32.
  qkv = x @ W_qkv; per-head logits = q k^T/sqrt(hd) + (j-i)*w_h;
  out = softmax(logits) @ v; y = concat_heads(out) @ W_proj.

Sharding: 2D (batch x head-group). Core c handles batch b=c//2 and head
group g=c%2 (8 of 16 heads, chosen by a window-overlap pairing).  Each
core computes a partial y (its heads' slice of W_proj rows); host sums
the two partials per batch.

Device algorithm (per core), all matmuls bf16 with fp32 PSUM accumulation:
  - host passes x^T (pre-transposed, bf16) so all matmuls contract over
    partition dim with no on-device transposes.
  - qT/kT [hd, T] per head and V [T, hd] come from one GEMM each.
  - logits are computed TRANSPOSED (j on partitions, i free):
      L^T[j,i] = sum_d kT[d,j] qT[d,i]
    so the softmax bias j*w_h is a per-partition constant: one ACT
    instruction does exp(scale*qk + (j*w_h - max_bias - BOUND)) straight
    out of PSUM.
  - V carries 64 extra all-ones columns per head, so attn@V leaves the
    softmax denominator REPLICATED on PSUM partitions 64..127.
  - out^T [d, i] is exactly the stationary layout the final projection
    needs; y partials stream out in bf16 (summed in fp32 on host).

Windowed softmax: weights decay like exp(-dist*|w_h|) away from the
bias-maximizing edge, so only j-chunks within dist <= WIN_MARGIN/|w_h| of
that edge contribute above ~1e-5 relative; other (j-chunk, head) work is
skipped.  Heads are PAIRED across the two core groups to maximize window
overlap (both cores run the same SPMD program over the pair's union).

Schedule (v1):
  - DMA arrival order wk, x0, wq, x1..x7, bias, wv, wp, striped
    round-robin over the 3 DMA-capable engine queues so chunks land in
    consumption order at the aggregate-HBM cadence.
  - short warmup covers the first chunk's landing and keeps the HAM
    clock-gate warm; kT waves + 4 early qT tiles consume chunks kc-major
    as they land so the PE never idles long enough to re-throttle.
  - attention block 0 (i in [0,1024)) weaves the remaining qT tiles as
    fillers; blocks [1024,1536) and [1536,2048) run slot-PAIRED logits:
    the even slot's kT/qT live on partitions 0..63 and the odd slot's on
    64..127, so the two 64-row logits matmuls co-run on disjoint PE
    quadrants (row groups), halving logits wall time.  Projection halves
    for t-chunks 0..13 fill these passes; only tch 14..15 remain as tail.
  - y is staged per t-chunk in a [128,1024] tile (half copied by scalar,
    half by vector) and shipped as ONE dma per chunk; the last two
    chunks' dmas are split across queues to shorten the drain.
"""

import numpy as np
import ml_dtypes

import concourse.bass as bass
import concourse.mybir as mybir
import concourse.tile as tile
from concourse import bacc
from concourse.bass_utils import run_bass_kernel_spmd

F32 = mybir.dt.float32
BF16 = mybir.dt.bfloat16
EXP = mybir.ActivationFunctionType.Exp
MULT = mybir.AluOpType.mult

B, T, D, H = 4, 2048, 1024, 16
HD = 64                      # head dim
N_CORES = 8
HL = 8                       # heads per core
PART = 128
TC = T // PART               # 16 j/t chunks
NT = 4                       # i-tiles
IT = T // NT                 # 512
IT2 = 2 * IT                 # 1024
DC = D // PART               # 8 model-dim K chunks
MC = (HL * HD) // PART       # 4 chunks of local head-dim (2 heads each)
SCALE = HD ** -0.5
B_QK = 24.0                  # safe upper bound for |q.k|*scale (randn data: ~8.3)
WIN_MARGIN = 8.9
WARMUP = 30                  # narrow (128-col) warmup matmuls covering the
                             # first x chunk's DMA landing
DEBUG_DUMP = False           # dump qT/kT/V/oT as extra outputs


def _window_chunks(w: float) -> frozenset:
    """128-aligned j-chunks whose softmax weight can matter, for slope w."""
    aw = abs(float(w))
    if aw < WIN_MARGIN / (T - 1):
        return frozenset(range(TC))
    d0 = int(np.ceil(WIN_MARGIN / aw))
    if w > 0:
        jmin = max(0, T - 1 - d0)
        return frozenset(range(jmin // PART, TC))
    jmax = min(T - 1, d0)
    return frozenset(range(0, jmax // PART + 1))


def _greedy_pair(items: list, sets: list) -> list:
    """Pair items greedily to minimize each pair's union size."""
    left = sorted(items, key=lambda i: -len(sets[i]))
    out = []
    while left:
        a = left.pop(0)
        b = min(left, key=lambda h: (len(sets[a] | sets[h]), len(sets[h])))
        left.remove(b)
        out.append((a, b))
    return out


def _plan(w: np.ndarray):
    """Head pairing + chunk windows from the actual W_rel."""
    cs = [_window_chunks(w[h]) for h in range(H)]
    pairs = _greedy_pair(list(range(H)), cs)          # (g0 head, g1 head) x 8
    pu = [cs[a] | cs[b] for a, b in pairs]
    mcg = _greedy_pair(list(range(len(pairs))), pu)   # pairs of pairs -> 4 mc
    slot_pairs = []
    for pa, pb in mcg:
        slot_pairs += [pairs[pa], pairs[pb]]
    jsets = [sorted(cs[a] | cs[b]) for a, b in slot_pairs]
    heads_g0 = [p[0] for p in slot_pairs]
    heads_g1 = [p[1] for p in slot_pairs]
    return jsets, heads_g0, heads_g1


def _runs(chunks) -> list[tuple[int, int]]:
    """Merge sorted chunk ids into contiguous [start_chunk, end_chunk) runs."""
    out = []
    for c in sorted(chunks):
        if out and out[-1][1] == c:
            out[-1][1] = c + 1
        else:
            out.append([c, c + 1])
    return [tuple(r) for r in out]


def _build_program(jsets: list[list[int]]):
    nc = bacc.Bacc("TRN2", target_bir_lowering=False, debug=False)

    xT_d = nc.dram_tensor("xT", (PART, DC * T), BF16, kind="ExternalInput")
    wq_d = nc.dram_tensor("wq", (PART, DC * HL * HD), BF16, kind="ExternalInput")
    wk_d = nc.dram_tensor("wk", (PART, DC * HL * HD), BF16, kind="ExternalInput")
    wv_d = nc.dram_tensor("wv", (PART, DC * HL * HD), BF16, kind="ExternalInput")
    wp_d = nc.dram_tensor("wp", (PART, MC * D), BF16, kind="ExternalInput")
    bias_d = nc.dram_tensor("biasT", (PART, TC * HL), F32, kind="ExternalInput")
    y_d = nc.dram_tensor("y", (T, D), BF16, kind="ExternalOutput")

    v_used = sorted({jc for js in jsets for jc in js})
    # mc order: heaviest pairs first, so light pairs (whose oT gates the
    # last projection matmuls) finish early in each pass.
    pair_order = sorted(
        range(MC), key=lambda m: -(len(jsets[2 * m]) + len(jsets[2 * m + 1]))
    )
    order = []
    for m in pair_order:
        a, b_ = 2 * m, 2 * m + 1
        order += [a, b_] if len(jsets[a]) >= len(jsets[b_]) else [b_, a]
    kt_runs = [_runs(set(jsets[2 * m]) | set(jsets[2 * m + 1])) for m in range(MC)]
    max_live_pt = max(len(js) for js in jsets)

    with tile.TileContext(nc) as tc:
        npt = min(max_live_pt + 3, 12)
        with (
            tc.tile_pool(name="sb", bufs=1) as cp,
            tc.tile_pool(name="ps", bufs=2, space=bass.MemorySpace.PSUM) as psp,
        ):
            xT = cp.tile([PART, DC, T], BF16, tag="xT")
            wq = cp.tile([PART, DC, HL * HD], BF16, tag="wq")
            wk = cp.tile([PART, DC, HL * HD], BF16, tag="wk")
            wv = cp.tile([PART, DC, HL * HD], BF16, tag="wv")
            wp = cp.tile([PART, MC, D], BF16, tag="wp")
            biasT = cp.tile([PART, TC, HL], F32, tag="biasT")
            qT = cp.tile([PART, MC, T], BF16, tag="qT")
            kT = cp.tile([PART, MC, T], BF16, tag="kT")
            V = cp.tile([PART, TC, HL * PART], BF16, tag="V")
            oT = cp.tile([PART, MC, T], BF16, tag="oT")

            # ---- PE warmup ----
            warm = cp.tile([PART, IT], BF16, tag="warm")
            nc.vector.memset(warm[:], 0.0)
            wps = psp.tile([PART, IT], F32, tag="acc", bufs=4)
            for i in range(WARMUP):
                nc.tensor.matmul(wps[:, 0:PART], warm[:, 0:PART],
                                 warm[:, 0:PART],
                                 start=(i == 0), stop=(i == WARMUP - 1))

            # ---- input DMAs: striped round-robin over the 3 queues in
            # consumption order: wk, x0, wq, x1..x7, bias, wv, wp.
            qeng = [nc.sync, nc.scalar, nc.gpsimd]
            W = HL * HD
            qctr = [0]

            def qnext():
                e = qeng[qctr[0] % 3]
                qctr[0] += 1
                return e

            for kc in range(DC):
                qnext().dma_start(wk[:, kc, :], wk_d.ap()[:, kc * W:(kc + 1) * W])
            for h in range(2):
                qnext().dma_start(
                    xT[:, 0, h * IT2:(h + 1) * IT2],
                    xT_d.ap()[:, h * IT2:(h + 1) * IT2])
            for kc in range(DC):
                qnext().dma_start(wq[:, kc, :], wq_d.ap()[:, kc * W:(kc + 1) * W])
            for kc in range(1, DC):
                for h in range(2):
                    qnext().dma_start(
                        xT[:, kc, h * IT2:(h + 1) * IT2],
                        xT_d.ap()[:, kc * T + h * IT2:kc * T + (h + 1) * IT2])
            qnext().dma_start(
                biasT[:].rearrange("p c h -> p (c h)"), bias_d.ap()[:])
            wv_flat = wv[:].rearrange("p c w -> p (c w)")
            for h in range(2):
                qnext().dma_start(
                    wv_flat[:, h * 4 * W:(h + 1) * 4 * W],
                    wv_d.ap()[:, h * 4 * W:(h + 1) * 4 * W])
            wp_flat = wp[:].rearrange("p c w -> p (c w)")
            for h in range(2):
                qnext().dma_start(
                    wp_flat[:, h * 2 * D:(h + 1) * 2 * D],
                    wp_d.ap()[:, h * 2 * D:(h + 1) * 2 * D])

            # ---- kT + early qT: kc-major waves so the PE consumes each
            # xT chunk the moment its DMA lands.  4 early qT groups (the
            # first mc's full i-range) are packed 2-per-lg-tile.
            spans = []                       # (mc, j0, j1)
            for mc in range(MC):
                for (c0, c1) in kt_runs[mc]:
                    j0, j1 = c0 * PART, c1 * PART
                    for s0 in range(j0, j1, IT):
                        spans.append((mc, s0, min(s0 + IT, j1)))
            mc0 = order[0] // 2
            qt_early = [(mc0, n5) for n5 in range(NT)]

            for w0 in range(0, len(spans), 4):
                wgrp = spans[w0:w0 + 4]
                tiles = [psp.tile([PART, IT], F32, tag="acc", bufs=4,
                                  name=f"kt_{w0}_{i}")
                         for i in range(len(wgrp))]
                overlap_q = []
                qtiles = []
                if w0 == 0:
                    overlap_q = qt_early
                    qtiles = [psp.tile([PART, IT2], F32, tag="lg",
                                       name=f"qte_{i}") for i in range(2)]
                for kc in range(DC):
                    for ti, (mc, j0, j1) in enumerate(wgrp):
                        nc.tensor.matmul(
                            tiles[ti][:, 0:j1 - j0],
                            wk[:, kc, mc * PART:(mc + 1) * PART],
                            xT[:, kc, j0:j1],
                            start=(kc == 0),
                            stop=(kc == DC - 1),
                        )
                    for qi, (mc, n5) in enumerate(overlap_q):
                        nc.tensor.matmul(
                            qtiles[qi // 2][:, (qi % 2) * IT:(qi % 2) * IT + IT],
                            wq[:, kc, mc * PART:(mc + 1) * PART],
                            xT[:, kc, n5 * IT:(n5 + 1) * IT],
                            start=(kc == 0),
                            stop=(kc == DC - 1),
                        )
                for ti, (mc, j0, j1) in enumerate(wgrp):
                    nc.vector.tensor_copy(
                        kT[:, mc, j0:j1], tiles[ti][:, 0:j1 - j0])
                for qi, (mc, n5) in enumerate(overlap_q):
                    nc.vector.tensor_copy(
                        qT[:, mc, n5 * IT:(n5 + 1) * IT],
                        qtiles[qi // 2][:, (qi % 2) * IT:(qi % 2) * IT + IT])

            # ---- V: [t, d'] = xT[:, t]^T @ Wv, 64 data + 64 ones per slot.
            # Runs right after the kT waves (x is fully resident by then).
            for jc in v_used:
                slots = [hh for hh in range(HL) if jc in jsets[hh]]
                for (s0, s1) in _runs(slots):
                    ps = psp.tile([PART, HL * HD], F32, tag="acc", bufs=4)
                    for kc in range(DC):
                        nc.tensor.matmul(
                            ps[:, 0:(s1 - s0) * HD],
                            xT[:, kc, jc * PART:(jc + 1) * PART],
                            wv[:, kc, s0 * HD:s1 * HD],
                            start=(kc == 0),
                            stop=(kc == DC - 1),
                        )
                    vv = V[:, jc, s0 * PART:s1 * PART].rearrange(
                        "p (h c) -> p h c", c=PART)
                    nc.vector.memset(vv[:, :, HD:PART], 1.0)
                    nc.vector.tensor_copy(
                        vv[:, :, 0:HD],
                        ps[:, 0:(s1 - s0) * HD].rearrange("p (h c) -> p h c", c=HD),
                    )

            # ---- remaining qT halves (pass-0 fillers) ----
            def emit_qT_half(mc, n5, h):
                c0 = n5 * IT + h * (IT // 2)
                ps = psp.tile([PART, IT], F32, tag="acc", bufs=4)
                for kc in range(DC):
                    nc.tensor.matmul(
                        ps[:, 0:IT // 2],
                        wq[:, kc, mc * PART:(mc + 1) * PART],
                        xT[:, kc, c0:c0 + IT // 2],
                        start=(kc == 0),
                        stop=(kc == DC - 1),
                    )
                nc.vector.tensor_copy(qT[:, mc, c0:c0 + IT // 2], ps[:, 0:IT // 2])

            mc_use = []
            for hh in order:
                if hh // 2 not in mc_use:
                    mc_use.append(hh // 2)
            for mc in range(MC):
                if mc not in mc_use:
                    mc_use.append(mc)
            # first-half (i<1024) tiles FIRST: pass 0's own logits consume
            # them mid-weave, and emission order is semantic order.
            filler_q = (
                [(mc, n5, h) for mc in mc_use for n5 in range(NT // 2)
                 for h in range(2) if mc != mc0]
                + [(mc, n5, h) for mc in mc_use for n5 in range(NT // 2, NT)
                   for h in range(2) if mc != mc0]
            )

            # ---- softmax epilogue (shared): denominator is replicated on
            # po partitions 64..127; copy-shift + reciprocal + multiply.
            def epilogue(hh, po, i0, width):
                mc = hh // 2
                pbase = (hh % 2) * HD
                d_sb = cp.tile([HD, IT], F32, tag="d", bufs=4)
                nc.scalar.copy(d_sb[:, 0:width], po[HD:PART, 0:width])
                r = cp.tile([HD, IT], F32, tag="r", bufs=4)
                nc.vector.reciprocal_approx_fast(r[:, 0:width], d_sb[:, 0:width])
                nc.vector.tensor_tensor(
                    oT[pbase:pbase + HD, mc, i0:i0 + width],
                    po[0:HD, 0:width], r[:, 0:width], MULT,
                )

            # ---- attention pass 0 (i in [0,1024)), unpaired ----
            def mk_logits(hh, jc, i0, width, state):
                def t():
                    lg = psp.tile([PART, IT2], F32, tag="lg")
                    for s0 in range(0, width, IT):
                        nc.tensor.matmul(
                            lg[:, s0:s0 + IT],
                            kT[(hh % 2) * HD:(hh % 2) * HD + HD, hh // 2,
                               jc * PART:(jc + 1) * PART],
                            qT[(hh % 2) * HD:(hh % 2) * HD + HD, hh // 2,
                               i0 + s0:i0 + s0 + IT],
                            start=True,
                            stop=True,
                        )
                    pt = cp.tile([PART, IT2], BF16, tag="pt", bufs=npt)
                    nc.scalar.activation(
                        pt[:, 0:width], lg[:, 0:width], EXP,
                        bias=biasT[:, jc, hh:hh + 1], scale=SCALE,
                    )
                    state[jc] = pt
                return t

            def mk_attnv(hh, jc, idx, its, i0, state):
                js = jsets[hh]

                def t():
                    if idx == 0:
                        state["po"] = {}
                        for (it, _) in its:
                            state["po"][it] = psp.tile(
                                [PART, IT], F32, tag="acc", bufs=4,
                                name=f"po_{hh}_{it}")
                    for (it, pt_off) in its:
                        nc.tensor.matmul(
                            state["po"][it],
                            V[:, jc, hh * PART:(hh + 1) * PART],
                            state[jc][:, pt_off:pt_off + IT],
                            start=(idx == 0),
                            stop=(idx == len(js) - 1),
                        )
                    if idx == len(js) - 1:
                        for (it, _) in its:
                            epilogue(hh, state["po"][it], i0 + it * IT, IT)
                return t

            def weave(lq, aq, fillers):
                stream = [lq[0]]
                for i in range(len(aq)):
                    if i + 1 < len(lq):
                        stream.append(lq[i + 1])
                    stream.append(aq[i])
                fi = 0
                for si, t in enumerate(stream):
                    t()
                    want = (si + 1) * len(fillers) // len(stream)
                    while fi < want:
                        fillers[fi]()
                        fi += 1

            def attention_pass0(fillers):
                lq, aq = [], []
                for hh in order:
                    state = {}
                    for idx, jc in enumerate(jsets[hh]):
                        lq.append(mk_logits(hh, jc, 0, IT2, state))
                        aq.append(mk_attnv(hh, jc, idx, [(0, 0), (1, IT)],
                                           0, state))
                weave(lq, aq, fillers)

            # ---- paired attention passes (width IT): even slot's logits
            # (partitions 0..63) and odd slot's (64..127) are emitted
            # back-to-back so the two 64-row matmuls co-run on disjoint
            # PE row-group quadrants.  Both land in one [128,1024] lg
            # tile (A in cols 0..511 = one bank, B in 512..1023).
            def mk_lg_pair(sA, sB, jcA, jcB, i0, width, stA, stB):
                def t():
                    lg = psp.tile([PART, IT2], F32, tag="lg")
                    for off, hh, jc in ((0, sA, jcA), (IT, sB, jcB)):
                        if jc is None:
                            continue
                        nc.tensor.matmul(
                            lg[:, off:off + width],
                            kT[(hh % 2) * HD:(hh % 2) * HD + HD, hh // 2,
                               jc * PART:(jc + 1) * PART],
                            qT[(hh % 2) * HD:(hh % 2) * HD + HD, hh // 2,
                               i0:i0 + width],
                            start=True,
                            stop=True,
                        )
                    for off, hh, jc, st in ((0, sA, jcA, stA),
                                            (IT, sB, jcB, stB)):
                        if jc is None:
                            continue
                        pt = cp.tile([PART, IT], BF16, tag="pt2", bufs=8)
                        nc.scalar.activation(
                            pt[:, 0:width], lg[:, off:off + width], EXP,
                            bias=biasT[:, jc, hh:hh + 1], scale=SCALE,
                        )
                        st[jc] = pt
                return t

            def mk_av_pair(sA, sB, jcA, jcB, idx, i0, width, stA, stB):
                def t():
                    for hh, jc, st in ((sA, jcA, stA), (sB, jcB, stB)):
                        if jc is None:
                            continue
                        js = jsets[hh]
                        if idx == 0:
                            st["po"] = psp.tile([PART, IT], F32, tag="acc",
                                                bufs=4, name=f"po_{hh}_{i0}")
                        nc.tensor.matmul(
                            st["po"][:, 0:width],
                            V[:, jc, hh * PART:(hh + 1) * PART],
                            st[jc][:, 0:width],
                            start=(idx == 0),
                            stop=(idx == len(js) - 1),
                        )
                        if idx == len(js) - 1:
                            epilogue(hh, st["po"], i0, width)
                return t

            def attention_pass_paired(i0, width, fillers):
                lq, aq = [], []
                for m in pair_order:
                    sA, sB = 2 * m, 2 * m + 1
                    if sA % 2:
                        sA, sB = sB, sA
                    jsA, jsB = jsets[sA], jsets[sB]
                    stA, stB = {}, {}
                    for idx in range(max(len(jsA), len(jsB))):
                        jcA = jsA[idx] if idx < len(jsA) else None
                        jcB = jsB[idx] if idx < len(jsB) else None
                        lq.append(mk_lg_pair(sA, sB, jcA, jcB, i0, width,
                                             stA, stB))
                        aq.append(mk_av_pair(sA, sB, jcA, jcB, idx, i0,
                                             width, stA, stB))
                weave(lq, aq, fillers)

            # ---- projection: per t-chunk, two 512-col halves into one
            # [128,1024] staging tile, one output DMA per chunk.
            y_tiles = {}
            ydma_q = [nc.gpsimd, nc.sync]

            def emit_proj_half(tch, no, split_dma=False):
                if no == 0:
                    y_tiles[tch] = cp.tile([PART, IT2], BF16, tag="y",
                                           bufs=3, name=f"y_{tch}")
                y_sb = y_tiles[tch]
                ps = psp.tile([PART, IT], F32, tag="acc", bufs=4)
                for idx, kc2 in enumerate(pair_order):
                    nc.tensor.matmul(
                        ps[:],
                        oT[:, kc2, tch * PART:(tch + 1) * PART],
                        wp[:, kc2, no * IT:(no + 1) * IT],
                        start=(idx == 0),
                        stop=(idx == MC - 1),
                    )
                if no == 0:
                    nc.scalar.copy(y_sb[:, 0:IT], ps[:])
                else:
                    nc.vector.tensor_copy(y_sb[:, IT:IT2], ps[:])
                    if not split_dma:
                        ydma_q[tch % 2].dma_start(
                            y_d.ap()[tch * PART:(tch + 1) * PART, :],
                            y_sb[:],
                        )
                    else:
                        for h, e in ((0, nc.sync), (1, nc.gpsimd)):
                            e.dma_start(
                                y_d.ap()[tch * PART:(tch + 1) * PART,
                                         h * IT:(h + 1) * IT],
                                y_sb[:, h * IT:(h + 1) * IT],
                            )

            # ---- schedule ----
            # (proj fillers may only reference oT rows COMPLETED by prior
            # passes: the PE queue is in-order, so a premature proj matmul
            # would deadlock against the pass's own epilogues.)
            attention_pass0(
                [(lambda mc=mc, n5=n5, h=h: emit_qT_half(mc, n5, h))
                 for (mc, n5, h) in filler_q])
            attention_pass_paired(
                2 * IT, IT,
                [(lambda t=t, no=no: emit_proj_half(t, no))
                 for t in range(8) for no in range(2)])
            attention_pass_paired(
                3 * IT, IT,
                [(lambda t=t, no=no: emit_proj_half(t, no))
                 for t in range(8, 12) for no in range(2)])
            # tail: t-chunks 12..15 (rows produced by the last pass); the
            # final two chunks' dmas are split across queues to drain fast
            for tch in range(12, 16):
                for no in range(2):
                    emit_proj_half(tch, no, split_dma=(tch >= 14))

            if DEBUG_DUMP:
                for name, t_, width in (
                    ("dbg_qT", qT, MC * T), ("dbg_kT", kT, MC * T),
                    ("dbg_V", V, TC * HL * PART), ("dbg_oT", oT, MC * T),
                ):
                    dd = nc.dram_tensor(name, (PART, width), BF16,
                                        kind="ExternalOutput")
                    nc.sync.dma_start(dd.ap()[:],
                                      t_[:].rearrange("p a b -> p (a b)"))

    nc.compile()
    return nc


def _prepare_inputs(x, W_qkv, W_proj, W_rel):
    x = np.asarray(x, dtype=np.float32)
    W_qkv = np.asarray(W_qkv, dtype=np.float32)
    W_proj = np.asarray(W_proj, dtype=np.float32)
    w = np.asarray(W_rel, dtype=np.float32).reshape(H)

    jsets, heads_g0, heads_g1 = _plan(w)

    def pmajor(a):
        """[C*128, M] -> [128, C*M] partition-major packing (bf16)."""
        cdim = a.shape[0] // PART
        return np.ascontiguousarray(
            a.reshape(cdim, PART, a.shape[1]).transpose(1, 0, 2).reshape(PART, -1)
        ).astype(ml_dtypes.bfloat16)

    j = np.arange(T, dtype=np.float64)
    in_maps = []
    for c in range(N_CORES):
        b, g = c // 2, c % 2
        heads = heads_g0 if g == 0 else heads_g1
        cw = w[heads].astype(np.float64)
        biasT = (
            j[:, None] * cw[None, :]
            - np.maximum(cw, 0.0)[None, :] * (T - 1)
            - B_QK
        ).astype(np.float32)  # [T, HL] per slot
        biasT_pm = np.ascontiguousarray(
            biasT.reshape(TC, PART, HL).transpose(1, 0, 2).reshape(PART, -1)
        )
        qcols = np.concatenate([np.arange(h * HD, (h + 1) * HD) for h in heads])
        in_maps.append({
            "xT": pmajor(x[b].T),
            "wq": pmajor(W_qkv[:, qcols]),
            "wk": pmajor(W_qkv[:, D + qcols]),
            "wv": pmajor(W_qkv[:, 2 * D + qcols]),
            "wp": pmajor(W_proj[qcols, :]),
            "biasT": biasT_pm,
        })
    return jsets, in_maps


def run(x, W_qkv, W_proj, W_rel, trace=False):
    jsets, in_maps = _prepare_inputs(x, W_qkv, W_proj, W_rel)
    nc = _build_program(jsets)
    res = run_bass_kernel_spmd(
        nc, in_maps, core_ids=list(range(N_CORES)), trace=trace
    )
    y = np.empty((B, T, D), dtype=np.float32)
    for b in range(B):
        y[b] = (res.results[2 * b]["y"].astype(np.float32)
                + res.results[2 * b + 1]["y"].astype(np.float32))
    return y, res


def kernel(x, W_qkv, W_proj, W_rel):
    y, _ = run(x, W_qkv, W_proj, W_rel, trace=False)
    return y


# revision 11
# speedup vs baseline: 1.0423x; 1.0423x over previous
"""Multi-head self-attention with linear relative-position bias on 8 trn2 cores.

Problem: B=4, T=2048, D=1024, H=16 heads (hd=64), fp32.
  qkv = x @ W_qkv; per-head logits = q k^T/sqrt(hd) + (j-i)*w_h;
  out = softmax(logits) @ v; y = concat_heads(out) @ W_proj.

Sharding: 2D (batch x head-group). Core c handles batch b=c//2 and head
group g=c%2 (8 of 16 heads, chosen by a window-overlap pairing).  Each
core computes a partial y (its heads' slice of W_proj rows); host sums
the two partials per batch.

Device algorithm (per core), all matmuls bf16 with fp32 PSUM accumulation:
  - host passes x^T (pre-transposed, bf16) so all matmuls contract over
    partition dim with no on-device transposes.
  - qT/kT [hd, T] per head and V [T, hd] come from one GEMM each.
  - logits are computed TRANSPOSED (j on partitions, i free):
      L^T[j,i] = sum_d kT[d,j] qT[d,i]
    so the softmax bias j*w_h is a per-partition constant: one ACT
    instruction does exp(scale*qk + (j*w_h - max_bias - BOUND)) straight
    out of PSUM.
  - V carries 64 extra all-ones columns per head, so attn@V leaves the
    softmax denominator REPLICATED on PSUM partitions 64..127.
  - out^T [d, i] is exactly the stationary layout the final projection
    needs; y partials stream out in bf16 (summed in fp32 on host).

Windowed softmax: weights decay like exp(-dist*|w_h|) away from the
bias-maximizing edge, so only j-chunks within dist <= WIN_MARGIN/|w_h| of
that edge contribute above ~1e-5 relative; other (j-chunk, head) work is
skipped.  Heads are PAIRED across the two core groups to maximize window
overlap (both cores run the same SPMD program over the pair's union).

Schedule (v1):
  - DMA arrival order wk, x0, wq, x1..x7, bias, wv, wp, striped
    round-robin over the 3 DMA-capable engine queues so chunks land in
    consumption order at the aggregate-HBM cadence.
  - short warmup covers the first chunk's landing and keeps the HAM
    clock-gate warm; kT waves + 4 early qT tiles consume chunks kc-major
    as they land so the PE never idles long enough to re-throttle.
  - attention block 0 (i in [0,1024)) weaves the remaining qT tiles as
    fillers; blocks [1024,1536) and [1536,2048) run slot-PAIRED logits:
    the even slot's kT/qT live on partitions 0..63 and the odd slot's on
    64..127, so the two 64-row logits matmuls co-run on disjoint PE
    quadrants (row groups), halving logits wall time.  Projection halves
    for t-chunks 0..13 fill these passes; only tch 14..15 remain as tail.
  - y is staged per t-chunk in a [128,1024] tile (half copied by scalar,
    half by vector) and shipped as ONE dma per chunk; the last two
    chunks' dmas are split across queues to shorten the drain.
"""

import numpy as np
import ml_dtypes

import concourse.bass as bass
import concourse.mybir as mybir
import concourse.tile as tile
from concourse import bacc
from concourse.bass_utils import run_bass_kernel_spmd

F32 = mybir.dt.float32
BF16 = mybir.dt.bfloat16
EXP = mybir.ActivationFunctionType.Exp
MULT = mybir.AluOpType.mult

B, T, D, H = 4, 2048, 1024, 16
HD = 64                      # head dim
N_CORES = 8
HL = 8                       # heads per core
PART = 128
TC = T // PART               # 16 j/t chunks
NT = 4                       # i-tiles
IT = T // NT                 # 512
IT2 = 2 * IT                 # 1024
DC = D // PART               # 8 model-dim K chunks
MC = (HL * HD) // PART       # 4 chunks of local head-dim (2 heads each)
SCALE = HD ** -0.5
B_QK = 24.0                  # safe upper bound for |q.k|*scale (randn data: ~8.3)
WIN_MARGIN = 8.9
WARMUP = 30                  # narrow (128-col) warmup matmuls covering the
                             # first x chunk's DMA landing
DEBUG_DUMP = False           # dump qT/kT/V/oT as extra outputs


def _window_chunks(w: float) -> frozenset:
    """128-aligned j-chunks whose softmax weight can matter, for slope w."""
    aw = abs(float(w))
    if aw < WIN_MARGIN / (T - 1):
        return frozenset(range(TC))
    d0 = int(np.ceil(WIN_MARGIN / aw))
    if w > 0:
        jmin = max(0, T - 1 - d0)
        return frozenset(range(jmin // PART, TC))
    jmax = min(T - 1, d0)
    return frozenset(range(0, jmax // PART + 1))


def _greedy_pair(items: list, sets: list) -> list:
    """Pair items greedily to minimize each pair's union size."""
    left = sorted(items, key=lambda i: -len(sets[i]))
    out = []
    while left:
        a = left.pop(0)
        b = min(left, key=lambda h: (len(sets[a] | sets[h]), len(sets[h])))
        left.remove(b)
        out.append((a, b))
    return out


def _plan(w: np.ndarray):
    """Head pairing + chunk windows from the actual W_rel."""
    cs = [_window_chunks(w[h]) for h in range(H)]
    pairs = _greedy_pair(list(range(H)), cs)          # (g0 head, g1 head) x 8
    pu = [cs[a] | cs[b] for a, b in pairs]
    mcg = _greedy_pair(list(range(len(pairs))), pu)   # pairs of pairs -> 4 mc
    slot_pairs = []
    for pa, pb in mcg:
        slot_pairs += [pairs[pa], pairs[pb]]
    jsets = [sorted(cs[a] | cs[b]) for a, b in slot_pairs]
    heads_g0 = [p[0] for p in slot_pairs]
    heads_g1 = [p[1] for p in slot_pairs]
    return jsets, heads_g0, heads_g1


def _runs(chunks) -> list[tuple[int, int]]:
    """Merge sorted chunk ids into contiguous [start_chunk, end_chunk) runs."""
    out = []
    for c in sorted(chunks):
        if out and out[-1][1] == c:
            out[-1][1] = c + 1
        else:
            out.append([c, c + 1])
    return [tuple(r) for r in out]


def _build_program(jsets: list[list[int]]):
    nc = bacc.Bacc("TRN2", target_bir_lowering=False, debug=False)

    xT_d = nc.dram_tensor("xT", (PART, DC * T), BF16, kind="ExternalInput")
    wq_d = nc.dram_tensor("wq", (PART, DC * HL * HD), BF16, kind="ExternalInput")
    wk_d = nc.dram_tensor("wk", (PART, DC * HL * HD), BF16, kind="ExternalInput")
    wv_d = nc.dram_tensor("wv", (PART, DC * HL * HD), BF16, kind="ExternalInput")
    wp_d = nc.dram_tensor("wp", (PART, MC * D), BF16, kind="ExternalInput")
    bias_d = nc.dram_tensor("biasT", (PART, TC * HL), F32, kind="ExternalInput")
    y_d = nc.dram_tensor("y", (T, D), BF16, kind="ExternalOutput")

    v_used = sorted({jc for js in jsets for jc in js})
    # mc order: heaviest pairs first, so light pairs (whose oT gates the
    # last projection matmuls) finish early in each pass.
    pair_order = sorted(
        range(MC), key=lambda m: -(len(jsets[2 * m]) + len(jsets[2 * m + 1]))
    )
    order = []
    for m in pair_order:
        a, b_ = 2 * m, 2 * m + 1
        order += [a, b_] if len(jsets[a]) >= len(jsets[b_]) else [b_, a]
    kt_runs = [_runs(set(jsets[2 * m]) | set(jsets[2 * m + 1])) for m in range(MC)]
    max_live_pt = max(len(js) for js in jsets)

    with tile.TileContext(nc) as tc:
        npt = min(max_live_pt + 3, 12)
        with (
            tc.tile_pool(name="sb", bufs=1) as cp,
            tc.tile_pool(name="ps", bufs=2, space=bass.MemorySpace.PSUM) as psp,
        ):
            xT = cp.tile([PART, DC, T], BF16, tag="xT")
            wq = cp.tile([PART, DC, HL * HD], BF16, tag="wq")
            wk = cp.tile([PART, DC, HL * HD], BF16, tag="wk")
            wv = cp.tile([PART, DC, HL * HD], BF16, tag="wv")
            wp = cp.tile([PART, MC, D], BF16, tag="wp")
            biasT = cp.tile([PART, TC, HL], F32, tag="biasT")
            qT = cp.tile([PART, MC, T], BF16, tag="qT")
            kT = cp.tile([PART, MC, T], BF16, tag="kT")
            V = cp.tile([PART, TC, HL * PART], BF16, tag="V")
            oT = cp.tile([PART, MC, T], BF16, tag="oT")

            # ---- PE warmup ----
            warm = cp.tile([PART, IT], BF16, tag="warm")
            nc.vector.memset(warm[:], 0.0)
            wps = psp.tile([PART, IT], F32, tag="acc", bufs=4)
            for i in range(WARMUP):
                nc.tensor.matmul(wps[:, 0:PART], warm[:, 0:PART],
                                 warm[:, 0:PART],
                                 start=(i == 0), stop=(i == WARMUP - 1))

            # ---- input DMAs: striped round-robin over the 3 queues in
            # consumption order: wk, x0, wq, x1..x7, bias, wv, wp.
            qeng = [nc.sync, nc.scalar, nc.gpsimd]
            W = HL * HD
            qctr = [0]

            def qnext():
                e = qeng[qctr[0] % 3]
                qctr[0] += 1
                return e

            for h in range(2):
                qnext().dma_start(
                    xT[:, 0, h * IT2:(h + 1) * IT2],
                    xT_d.ap()[:, h * IT2:(h + 1) * IT2])
            for kc in range(0, DC, 2):
                qnext().dma_start(
                    wq[:, kc:kc + 2, :].rearrange("p c w -> p (c w)"),
                    wq_d.ap()[:, kc * W:(kc + 2) * W])
            for kc in range(1, DC):
                for h in range(2):
                    qnext().dma_start(
                        xT[:, kc, h * IT2:(h + 1) * IT2],
                        xT_d.ap()[:, kc * T + h * IT2:kc * T + (h + 1) * IT2])
            for kc in range(0, DC, 2):
                qnext().dma_start(
                    wk[:, kc:kc + 2, :].rearrange("p c w -> p (c w)"),
                    wk_d.ap()[:, kc * W:(kc + 2) * W])
            qnext().dma_start(
                biasT[:].rearrange("p c h -> p (c h)"), bias_d.ap()[:])
            wv_flat = wv[:].rearrange("p c w -> p (c w)")
            for h in range(2):
                qnext().dma_start(
                    wv_flat[:, h * 4 * W:(h + 1) * 4 * W],
                    wv_d.ap()[:, h * 4 * W:(h + 1) * 4 * W])
            wp_flat = wp[:].rearrange("p c w -> p (c w)")
            for h in range(2):
                qnext().dma_start(
                    wp_flat[:, h * 2 * D:(h + 1) * 2 * D],
                    wp_d.ap()[:, h * 2 * D:(h + 1) * 2 * D])

            # ---- early qT: a kc-major wave of EIGHT 512-col accumulation
            # groups (all 8 PSUM banks: 4 acc tiles + 2 lg tiles holding 2
            # groups each) for the first two mc's full i-range, so the PE
            # consumes each xT chunk the moment its DMA lands and never
            # idles long enough for the HAM clock-gate to re-throttle.
            mc_use = []
            for hh in order:
                if hh // 2 not in mc_use:
                    mc_use.append(hh // 2)
            for mc in range(MC):
                if mc not in mc_use:
                    mc_use.append(mc)
            qt_early = [(mc_use[0], n5) for n5 in range(NT)] + \
                       [(mc_use[1], n5) for n5 in range(NT)]
            eacc = [psp.tile([PART, IT], F32, tag="acc", bufs=4,
                             name=f"qte_a{i}") for i in range(4)]
            elg = [psp.tile([PART, IT2], F32, tag="lg",
                            name=f"qte_l{i}") for i in range(2)]
            egrp = [eacc[i][:, 0:IT] for i in range(4)] + \
                   [elg[i // 2][:, (i % 2) * IT:(i % 2) * IT + IT]
                    for i in range(4)]
            for kc in range(DC):
                for qi, (mc, n5) in enumerate(qt_early):
                    nc.tensor.matmul(
                        egrp[qi],
                        wq[:, kc, mc * PART:(mc + 1) * PART],
                        xT[:, kc, n5 * IT:(n5 + 1) * IT],
                        start=(kc == 0),
                        stop=(kc == DC - 1),
                    )
            for qi, (mc, n5) in enumerate(qt_early):
                nc.vector.tensor_copy(
                    qT[:, mc, n5 * IT:(n5 + 1) * IT], egrp[qi])

            # ---- kT: windowed j-runs, kc-major waves (x resident by now)
            spans = []                       # (mc, j0, j1)
            for mc in range(MC):
                for (c0, c1) in kt_runs[mc]:
                    j0, j1 = c0 * PART, c1 * PART
                    for s0 in range(j0, j1, IT):
                        spans.append((mc, s0, min(s0 + IT, j1)))
            for w0 in range(0, len(spans), 4):
                wgrp = spans[w0:w0 + 4]
                tiles = [psp.tile([PART, IT], F32, tag="acc", bufs=4,
                                  name=f"kt_{w0}_{i}")
                         for i in range(len(wgrp))]
                for kc in range(DC):
                    for ti, (mc, j0, j1) in enumerate(wgrp):
                        nc.tensor.matmul(
                            tiles[ti][:, 0:j1 - j0],
                            wk[:, kc, mc * PART:(mc + 1) * PART],
                            xT[:, kc, j0:j1],
                            start=(kc == 0),
                            stop=(kc == DC - 1),
                        )
                for ti, (mc, j0, j1) in enumerate(wgrp):
                    nc.vector.tensor_copy(
                        kT[:, mc, j0:j1], tiles[ti][:, 0:j1 - j0])

            # ---- V: [t, d'] = xT[:, t]^T @ Wv, 64 data + 64 ones per slot.
            # Runs right after the kT waves (x is fully resident by then).
            for jc in v_used:
                slots = [hh for hh in range(HL) if jc in jsets[hh]]
                for (s0, s1) in _runs(slots):
                    ps = psp.tile([PART, HL * HD], F32, tag="acc", bufs=4)
                    for kc in range(DC):
                        nc.tensor.matmul(
                            ps[:, 0:(s1 - s0) * HD],
                            xT[:, kc, jc * PART:(jc + 1) * PART],
                            wv[:, kc, s0 * HD:s1 * HD],
                            start=(kc == 0),
                            stop=(kc == DC - 1),
                        )
                    vv = V[:, jc, s0 * PART:s1 * PART].rearrange(
                        "p (h c) -> p h c", c=PART)
                    nc.vector.memset(vv[:, :, HD:PART], 1.0)
                    nc.vector.tensor_copy(
                        vv[:, :, 0:HD],
                        ps[:, 0:(s1 - s0) * HD].rearrange("p (h c) -> p h c", c=HD),
                    )

            # ---- remaining qT tiles (pass-0 fillers) ----
            def emit_qT_tile(mc, n5):
                ps = psp.tile([PART, IT], F32, tag="acc", bufs=4)
                for kc in range(DC):
                    nc.tensor.matmul(
                        ps[:],
                        wq[:, kc, mc * PART:(mc + 1) * PART],
                        xT[:, kc, n5 * IT:(n5 + 1) * IT],
                        start=(kc == 0),
                        stop=(kc == DC - 1),
                    )
                nc.vector.tensor_copy(qT[:, mc, n5 * IT:(n5 + 1) * IT], ps[:])

            # first-half (i<1024) tiles FIRST: pass 0's own logits consume
            # them mid-weave, and emission order is semantic order.
            mc_late = mc_use[2:]
            filler_q = (
                [(mc, n5) for mc in mc_late for n5 in range(NT // 2)]
                + [(mc, n5) for mc in mc_late for n5 in range(NT // 2, NT)]
            )

            # ---- softmax epilogue (shared): denominator is replicated on
            # po partitions 64..127; copy-shift + reciprocal + multiply.
            def epilogue(hh, po, i0, width):
                mc = hh // 2
                pbase = (hh % 2) * HD
                d_sb = cp.tile([HD, IT], F32, tag="d", bufs=4)
                nc.scalar.copy(d_sb[:, 0:width], po[HD:PART, 0:width])
                r = cp.tile([HD, IT], F32, tag="r", bufs=4)
                nc.vector.reciprocal_approx_fast(r[:, 0:width], d_sb[:, 0:width])
                nc.vector.tensor_tensor(
                    oT[pbase:pbase + HD, mc, i0:i0 + width],
                    po[0:HD, 0:width], r[:, 0:width], MULT,
                )

            # ---- attention pass 0 (i in [0,1024)), unpaired ----
            def mk_logits(hh, jc, i0, width, state):
                def t():
                    lg = psp.tile([PART, IT2], F32, tag="lg")
                    for s0 in range(0, width, IT):
                        nc.tensor.matmul(
                            lg[:, s0:s0 + IT],
                            kT[(hh % 2) * HD:(hh % 2) * HD + HD, hh // 2,
                               jc * PART:(jc + 1) * PART],
                            qT[(hh % 2) * HD:(hh % 2) * HD + HD, hh // 2,
                               i0 + s0:i0 + s0 + IT],
                            start=True,
                            stop=True,
                        )
                    pt = cp.tile([PART, IT2], BF16, tag="pt", bufs=npt)
                    nc.scalar.activation(
                        pt[:, 0:width], lg[:, 0:width], EXP,
                        bias=biasT[:, jc, hh:hh + 1], scale=SCALE,
                    )
                    state[jc] = pt
                return t

            def mk_attnv(hh, jc, idx, its, i0, state):
                js = jsets[hh]

                def t():
                    if idx == 0:
                        state["po"] = {}
                        for (it, _) in its:
                            state["po"][it] = psp.tile(
                                [PART, IT], F32, tag="acc", bufs=4,
                                name=f"po_{hh}_{it}")
                    for (it, pt_off) in its:
                        nc.tensor.matmul(
                            state["po"][it],
                            V[:, jc, hh * PART:(hh + 1) * PART],
                            state[jc][:, pt_off:pt_off + IT],
                            start=(idx == 0),
                            stop=(idx == len(js) - 1),
                        )
                    if idx == len(js) - 1:
                        for (it, _) in its:
                            epilogue(hh, state["po"][it], i0 + it * IT, IT)
                return t

            def weave(lq, aq, fillers):
                stream = [lq[0]]
                for i in range(len(aq)):
                    if i + 1 < len(lq):
                        stream.append(lq[i + 1])
                    stream.append(aq[i])
                fi = 0
                for si, t in enumerate(stream):
                    t()
                    want = (si + 1) * len(fillers) // len(stream)
                    while fi < want:
                        fillers[fi]()
                        fi += 1

            def attention_pass0(fillers):
                lq, aq = [], []
                for hh in order:
                    state = {}
                    for idx, jc in enumerate(jsets[hh]):
                        lq.append(mk_logits(hh, jc, 0, IT2, state))
                        aq.append(mk_attnv(hh, jc, idx, [(0, 0), (1, IT)],
                                           0, state))
                weave(lq, aq, fillers)

            # ---- paired attention passes (width IT): even slot's logits
            # (partitions 0..63) and odd slot's (64..127) are emitted
            # back-to-back so the two 64-row matmuls co-run on disjoint
            # PE row-group quadrants.  Both land in one [128,1024] lg
            # tile (A in cols 0..511 = one bank, B in 512..1023).
            def mk_lg_pair(sA, sB, jcA, jcB, i0, width, stA, stB):
                def t():
                    lg = psp.tile([PART, IT2], F32, tag="lg")
                    for off, hh, jc in ((0, sA, jcA), (IT, sB, jcB)):
                        if jc is None:
                            continue
                        nc.tensor.matmul(
                            lg[:, off:off + width],
                            kT[(hh % 2) * HD:(hh % 2) * HD + HD, hh // 2,
                               jc * PART:(jc + 1) * PART],
                            qT[(hh % 2) * HD:(hh % 2) * HD + HD, hh // 2,
                               i0:i0 + width],
                            start=True,
                            stop=True,
                        )
                    for off, hh, jc, st in ((0, sA, jcA, stA),
                                            (IT, sB, jcB, stB)):
                        if jc is None:
                            continue
                        pt = cp.tile([PART, IT], BF16, tag="pt2", bufs=8)
                        nc.scalar.activation(
                            pt[:, 0:width], lg[:, off:off + width], EXP,
                            bias=biasT[:, jc, hh:hh + 1], scale=SCALE,
                        )
                        st[jc] = pt
                return t

            def mk_av_pair(sA, sB, jcA, jcB, idx, i0, width, stA, stB):
                def t():
                    for hh, jc, st in ((sA, jcA, stA), (sB, jcB, stB)):
                        if jc is None:
                            continue
                        js = jsets[hh]
                        if idx == 0:
                            st["po"] = psp.tile([PART, IT], F32, tag="acc",
                                                bufs=4, name=f"po_{hh}_{i0}")
                        nc.tensor.matmul(
                            st["po"][:, 0:width],
                            V[:, jc, hh * PART:(hh + 1) * PART],
                            st[jc][:, 0:width],
                            start=(idx == 0),
                            stop=(idx == len(js) - 1),
                        )
                        if idx == len(js) - 1:
                            epilogue(hh, st["po"], i0, width)
                return t

            def attention_pass_paired(i0, width, fillers):
                lq, aq = [], []
                for m in pair_order:
                    sA, sB = 2 * m, 2 * m + 1
                    if sA % 2:
                        sA, sB = sB, sA
                    jsA, jsB = jsets[sA], jsets[sB]
                    stA, stB = {}, {}
                    for idx in range(max(len(jsA), len(jsB))):
                        jcA = jsA[idx] if idx < len(jsA) else None
                        jcB = jsB[idx] if idx < len(jsB) else None
                        lq.append(mk_lg_pair(sA, sB, jcA, jcB, i0, width,
                                             stA, stB))
                        aq.append(mk_av_pair(sA, sB, jcA, jcB, idx, i0,
                                             width, stA, stB))
                weave(lq, aq, fillers)

            # ---- projection: per t-chunk, two 512-col halves into one
            # [128,1024] staging tile, one output DMA per chunk.
            y_tiles = {}
            ydma_q = [nc.gpsimd, nc.sync]

            def emit_proj_half(tch, no, split_dma=False):
                if no == 0:
                    y_tiles[tch] = cp.tile([PART, IT2], BF16, tag="y",
                                           bufs=3, name=f"y_{tch}")
                y_sb = y_tiles[tch]
                ps = psp.tile([PART, IT], F32, tag="acc", bufs=4)
                for idx, kc2 in enumerate(pair_order):
                    nc.tensor.matmul(
                        ps[:],
                        oT[:, kc2, tch * PART:(tch + 1) * PART],
                        wp[:, kc2, no * IT:(no + 1) * IT],
                        start=(idx == 0),
                        stop=(idx == MC - 1),
                    )
                if no == 0:
                    nc.scalar.copy(y_sb[:, 0:IT], ps[:])
                else:
                    nc.vector.tensor_copy(y_sb[:, IT:IT2], ps[:])
                    if not split_dma:
                        ydma_q[tch % 2].dma_start(
                            y_d.ap()[tch * PART:(tch + 1) * PART, :],
                            y_sb[:],
                        )
                    else:
                        for h, e in ((0, nc.sync), (1, nc.gpsimd)):
                            e.dma_start(
                                y_d.ap()[tch * PART:(tch + 1) * PART,
                                         h * IT:(h + 1) * IT],
                                y_sb[:, h * IT:(h + 1) * IT],
                            )

            # ---- schedule ----
            # (proj fillers may only reference oT rows COMPLETED by prior
            # passes: the PE queue is in-order, so a premature proj matmul
            # would deadlock against the pass's own epilogues.)
            attention_pass0(
                [(lambda mc=mc, n5=n5: emit_qT_tile(mc, n5))
                 for (mc, n5) in filler_q])
            attention_pass_paired(
                2 * IT, IT,
                [(lambda t=t, no=no: emit_proj_half(t, no))
                 for t in range(8) for no in range(2)])
            attention_pass_paired(
                3 * IT, IT,
                [(lambda t=t, no=no: emit_proj_half(t, no))
                 for t in range(8, 12) for no in range(2)])
            # tail: t-chunks 12..15 (rows produced by the last pass); the
            # final two chunks' dmas are split across queues to drain fast
            for tch in range(12, 16):
                for no in range(2):
                    emit_proj_half(tch, no, split_dma=(tch >= 14))

            if DEBUG_DUMP:
                for name, t_, width in (
                    ("dbg_qT", qT, MC * T), ("dbg_kT", kT, MC * T),
                    ("dbg_V", V, TC * HL * PART), ("dbg_oT", oT, MC * T),
                ):
                    dd = nc.dram_tensor(name, (PART, width), BF16,
                                        kind="ExternalOutput")
                    nc.sync.dma_start(dd.ap()[:],
                                      t_[:].rearrange("p a b -> p (a b)"))

    nc.compile()
    return nc


def _prepare_inputs(x, W_qkv, W_proj, W_rel):
    x = np.asarray(x, dtype=np.float32)
    W_qkv = np.asarray(W_qkv, dtype=np.float32)
    W_proj = np.asarray(W_proj, dtype=np.float32)
    w = np.asarray(W_rel, dtype=np.float32).reshape(H)

    jsets, heads_g0, heads_g1 = _plan(w)

    def pmajor(a):
        """[C*128, M] -> [128, C*M] partition-major packing (bf16)."""
        cdim = a.shape[0] // PART
        return np.ascontiguousarray(
            a.reshape(cdim, PART, a.shape[1]).transpose(1, 0, 2).reshape(PART, -1)
        ).astype(ml_dtypes.bfloat16)

    j = np.arange(T, dtype=np.float64)
    in_maps = []
    for c in range(N_CORES):
        b, g = c // 2, c % 2
        heads = heads_g0 if g == 0 else heads_g1
        cw = w[heads].astype(np.float64)
        biasT = (
            j[:, None] * cw[None, :]
            - np.maximum(cw, 0.0)[None, :] * (T - 1)
            - B_QK
        ).astype(np.float32)  # [T, HL] per slot
        biasT_pm = np.ascontiguousarray(
            biasT.reshape(TC, PART, HL).transpose(1, 0, 2).reshape(PART, -1)
        )
        qcols = np.concatenate([np.arange(h * HD, (h + 1) * HD) for h in heads])
        in_maps.append({
            "xT": pmajor(x[b].T),
            "wq": pmajor(W_qkv[:, qcols]),
            "wk": pmajor(W_qkv[:, D + qcols]),
            "wv": pmajor(W_qkv[:, 2 * D + qcols]),
            "wp": pmajor(W_proj[qcols, :]),
            "biasT": biasT_pm,
        })
    return jsets, in_maps


def run(x, W_qkv, W_proj, W_rel, trace=False):
    jsets, in_maps = _prepare_inputs(x, W_qkv, W_proj, W_rel)
    nc = _build_program(jsets)
    res = run_bass_kernel_spmd(
        nc, in_maps, core_ids=list(range(N_CORES)), trace=trace
    )
    y = np.empty((B, T, D), dtype=np.float32)
    for b in range(B):
        y[b] = (res.results[2 * b]["y"].astype(np.float32)
                + res.results[2 * b + 1]["y"].astype(np.float32))
    return y, res


def kernel(x, W_qkv, W_proj, W_rel):
    y, _ = run(x, W_qkv, W_proj, W_rel, trace=False)
    return y
